# revision 29
# baseline (speedup 1.0000x reference)
"""Multi-head causal attention (B=2, L=2048, H=2048, NH=16) on 8 Trainium2
NeuronCores.

Sharding: tensor-parallel over heads — core c computes heads {2c, 2c+1}.
Each core:
  phase 1: q/k/v projections for its 256 output dims (contract over H=2048)
  phase 2: causal attention for its 2 heads + its partial o-projection
Host: transposes/casts inputs to bf16, sums the 8 partial bf16
o-projection outputs, and transposes back.

All matmuls run in bf16 (PE 1 cycle/row, FWL weight loads) with fp32 PSUM
accumulation; measured end-to-end rel absmax ~5e-3 vs the fp32 reference
(gate is 2e-2).

Softmax denominator never rides the PE per j-tile: exp tiles accumulate on
the DVE into sumex, then one ones-matmul per (chunk, head)
partition-reduces sumex into a full 128-partition broadcast so the
reciprocal + normalize are short DVE ops.

PE never sits at the head of its FIFO waiting on a slow producer:
  - per-(chunk, head) normalize (reciprocal + multiply) is a deferred
    stage drained one j-tile after its inputs started
  - o-projection of chunk N drains interleaved between the score and AV
    matmuls of chunk N+1 (paced by j-tile stamps), covering the
    score->exp->mask->AV latency
  - 28 warmup matmuls on a memset tile bridge the ~20us startup DMA gate
    (x + Wq delivery, behind the scheduler's batched group waits) so the
    HAM clock gate is warm when real work arrives
  - phase-2's first chunk (b1, 0:256) is computed mid-phase-1, hiding its
    exp/mask/DVE work under the dense projection stream; the trailing
    (b1, 256:512) chunk runs last at half width for a small exposed tail
"""

import heapq
import os
import sys

if "/opt/trn_rl_repo" not in sys.path:
    sys.path.insert(0, "/opt/trn_rl_repo")

import numpy as np

from concourse import bacc, mybir, tile  # noqa: E402
from concourse.bass_utils import run_bass_kernel_spmd  # noqa: E402

BF16 = mybir.dt.bfloat16
F32R = mybir.dt.float32r
F32 = mybir.dt.float32

N_CORES = 8
B, L, H, NH = 2, 2048, 2048, 16
DH = H // NH                      # 128
BL = B * L                        # 4096
HPC = NH // N_CORES               # heads per core = 2
OPC = HPC * DH                    # output dims per core = 256
HT = H // 128                     # 16 h-tiles (contraction)
IC1 = 512                         # phase-1 i-chunk width
N_IC1 = BL // IC1                 # 8
IC2 = 512                         # phase-2 i-chunk width
SCALE = 1.0 / float(np.sqrt(DH))

# phase-2 chunks (batch, start, width). The (b1, 0) region is split in two:
# its first half leads (the no-filler chunk is the tiny one, and its o-proj
# then fills the first big chunk), its second half trails (half-width
# un-hideable o-projection tail; 128-wide would write 256B DMA lines —
# below the 512B line-rate threshold)
CHUNKS = [(1, 0, 256), (0, 0, 512), (0, 512, 512), (0, 1024, 512),
          (0, 1536, 512), (1, 512, 512), (1, 1024, 512), (1, 1536, 512),
          (1, 256, 256)]

LAST_EXEC_NS = None


def _build():
    nc = bacc.Bacc(None, target_bir_lowering=False, debug=False)

    # DRAM layouts are packed host-side so every DMA is ~128 runs of
    # 2-4KB (one per partition): descriptor generation (DIRECT2D on the
    # SP/ACT sequencers, ~3.5us per MB for 512B runs) otherwise paces
    # the whole startup and occupies the ACT sequencer mid-kernel.
    #   xt:  [c, q, p, t, f] = xt2d[q*1024 + t*128 + p, c*256 + f]
    #   wq/wk/wv: [q, p, t, f] = w2d[q*1024 + t*128 + p, f]
    #   wo:  [q, p, f]         = wo2d[q*128 + p, f]
    #   out: [c, g, p, t, f]  -> out2d[g*512 + t*128 + p, c*512 + f]
    xt = nc.declare_dram_parameter("xt", [BL // 256, 2, 128, HT // 2, 256],
                                   BF16, isOutput=False)
    wq = nc.declare_dram_parameter("wq", [2, 128, HT // 2, OPC], BF16,
                                   isOutput=False)
    wk = nc.declare_dram_parameter("wk", [2, 128, HT // 2, OPC], BF16,
                                   isOutput=False)
    wv = nc.declare_dram_parameter("wv", [2, 128, HT // 2, OPC], BF16,
                                   isOutput=False)
    wo = nc.declare_dram_parameter("wo", [2, 128, H], BF16, isOutput=False)
    out = nc.declare_dram_parameter("out", [BL // 512, 4, 128, 4, 512],
                                    BF16, isOutput=True)

    with tile.TileContext(nc) as tc:
        with tc.tile_pool(name="persist", bufs=1) as persist, \
             tc.tile_pool(name="psum", bufs=8, space="PSUM") as psum, \
             tc.tile_pool(name="wpool", bufs=1) as wpool, \
             tc.tile_pool(name="xpool", bufs=2) as xpool, \
             tc.tile_pool(name="misc1", bufs=1) as misc1, \
             tc.tile_pool(name="wo_pool", bufs=1) as wo_pool, \
             tc.tile_pool(name="exp_pool", bufs=8) as exp_pool, \
             tc.tile_pool(name="sm_pool", bufs=2) as sm_pool, \
             tc.tile_pool(name="mst_pool", bufs=3) as mst_pool, \
             tc.tile_pool(name="ob_pool", bufs=2) as ob_pool:
            qt_sb = persist.tile([128, HPC, BL], BF16, tag="qt")
            kt_sb = persist.tile([128, HPC, BL], BF16, tag="kt")
            v_sb = persist.tile([128, BL // 128, OPC], BF16, tag="v")
            ones_sb = persist.tile([128, 128], BF16, tag="ones")

            # ---- warmup: bridge the DMA ramp, trip the HAM clock gate ----
            warm = misc1.tile([128, 512], BF16, tag="warm")
            nc.vector.memset(warm[:, :], 1.0)
            wps = psum.tile([128, 512], F32, tag="bank", name="wps")
            for i in range(28):
                nc.tensor.matmul(wps[:, :], warm[:, :128], warm[:, :],
                                 start=(i == 0), stop=(i == 27))
            sink = misc1.tile([1, 512], F32, tag="sink")
            nc.vector.tensor_copy(sink[:, :], wps[0:1, :])

            # ---------------- startup DMAs ----------------
            wq_sb = wpool.tile([128, HT, OPC], BF16, tag="wq")
            wk_sb = wpool.tile([128, HT, OPC], BF16, tag="wk")
            wv_sb = wpool.tile([128, HT, OPC], BF16, tag="wv")
            wo_sb = wo_pool.tile([128, HPC, H], BF16, tag="wo")

            def dma_w(dst3, src):
                # weight tile [128, HT, OPC]: t-halves ride the two HW
                # queues; both sides are 4KB-per-partition contiguous
                nc.sync.dma_start(out=dst3[:, :HT // 2, :], in_=src[0])
                nc.scalar.dma_start(out=dst3[:, HT // 2:, :], in_=src[1])

            def dma_x(xch, c0, w):
                # xch layout [128, c(2), HT, 256]: per (partition, c)
                # the t*f block is 4KB contiguous on both sides
                nblk = w // 256
                c = c0 // 256
                src = xt[c:c + nblk]
                nc.sync.dma_start(
                    out=xch[:, :nblk, :HT // 2, :],
                    in_=src[:, 0].rearrange("c p t f -> p c t f"))
                nc.scalar.dma_start(
                    out=xch[:, :nblk, HT // 2:, :],
                    in_=src[:, 1].rearrange("c p t f -> p c t f"))

            # Phase-1 i-chunks: the first two are 256-wide so the very
            # first accumulation group's batched-wait gate covers only
            # 1MB of x (+1MB wq) instead of 2MB. The rest are 512-wide.
            P1 = [(0, 256), (256, 256)] + [
                (c, 512) for c in range(512, BL, 512)]
            xchs = {}
            xchs[0] = xpool.tile([128, 2, HT, 256], BF16, tag="xch",
                                 name="xch")
            dma_x(xchs[0], 0, 256)
            dma_w(wq_sb, wq)
            xchs[1] = xpool.tile([128, 2, HT, 256], BF16, tag="xch",
                                 name="xch")
            dma_x(xchs[1], 256, 256)
            dma_w(wk_sb, wk)
            dma_w(wv_sb, wv)

            ones_f = misc1.tile([128, 128], F32)
            nc.vector.memset(ones_f[:, :], 1.0)
            nc.vector.tensor_copy(ones_sb[:, :], ones_f[:, :])

            # -------- phase-2 machinery (used mid-phase-1 too) --------
            # deferred-work min-heap keyed by (ready_at_counter, seq):
            # items drain between a j-tile's score and AV matmuls so the
            # PE FIFO never stalls on a producer that just started
            pend = []
            seq_counter = [0]

            def push(ready_at, fn):
                heapq.heappush(pend, (ready_at, seq_counter[0], fn))
                seq_counter[0] += 1

            def drain(counter):
                while pend and pend[0][0] <= counter:
                    heapq.heappop(pend)[2]()

            def emit_oproj_ot(mst, obuf, w, ot, tail=False):
                op = psum.tile([128, w], F32, tag="bank", name="op",
                               padded_shape=[128, IC2])
                for hh in range(HPC):
                    nc.tensor.matmul(
                        op[:, :],
                        wo_sb[:, hh, ot * 128:(ot + 1) * 128],
                        mst[:, hh, :w],
                        start=(hh == 0), stop=(hh == HPC - 1))
                # steady state all PSUM->SBUF copies ride the DVE (ACT is
                # saturated by exp); in the tail exp is done, so alternate
                # DVE/ACT to halve the exposed copy chain
                if tail and ot % 2 == 1:
                    nc.scalar.copy(obuf[:, ot, :w], op[:, :])
                else:
                    nc.vector.tensor_copy(obuf[:, ot, :w], op[:, :])

            def emit_out_dma(obuf, gio, w, g, eng=None):
                # one grouped DMA per 4 o-tiles; sync queue by default
                # (scalar stays free for exp). Packed out layout: both
                # sides 4KB contiguous per partition for w=512.
                iC, f0 = gio // 512, gio % 512
                (eng or nc.sync).dma_start(
                    out=out[iC, g, :, :, f0:f0 + w],
                    in_=obuf[:, g * 4:(g + 1) * 4, :w])

            def make_norm(rs, mx, mst, h, w):
                def fn():
                    rec = sm_pool.tile([128, IC2], F32, tag="rec",
                                       name="rec")
                    nc.vector.reciprocal_approx_fast(
                        out=rec[:, :w], in_=rs[:, :])
                    nc.vector.tensor_mul(mst[:, h, :w], mx[:, :],
                                         rec[:, :w])
                return fn

            state = {"counter": 0, "pending": None}

            # process_chunk is a GENERATOR: it yields after each j-tile
            # so an attention chunk can be pumped piecewise between the
            # matmul groups of a phase-1 chunk — its exp/mask/normalize
            # chains then hide under dense dependency-free PE work
            # instead of running exposed at a chunk boundary.
            def process_chunk(b, i0, w, late=False, inline_norm=False):
                gio = b * L + i0
                njt = (i0 + w) // 128
                total_jts = HPC * njt
                mst = mst_pool.tile([128, HPC, IC2], BF16, tag="mst",
                                    name="mst")
                obuf = ob_pool.tile([128, H // 128, IC2], BF16,
                                    tag="obuf", name="obuf")
                # pace the previous chunk's o-proj across this chunk's
                # j-tiles; stamps start at +2 so the previous chunk's
                # deferred h1 normalize (stamp +1) always emits its mst
                # mul first. The last item also fires the out DMAs.
                if state["pending"] is not None:
                    pmst, pobuf, pgio, pw = state["pending"]
                    ng = H // 128
                    for i in range(ng):
                        ready = state["counter"] + 2 + (
                            i * max(total_jts - 2, 1)) // ng

                        def fi(i=i, pmst=pmst, pobuf=pobuf, pgio=pgio,
                               pw=pw, late=late):
                            emit_oproj_ot(pmst, pobuf, pw, i, tail=late)
                            if i == ng - 1:
                                for g in range(4):
                                    emit_out_dma(pobuf, pgio, pw, g)
                        push(ready, fi)
                def emit_csav(h, jt, f0, ex):
                    # colsum on the PE: all-ones stationary gives the
                    # denominator broadcast to all partitions
                    nc.tensor.matmul(
                        rs[:, f0:], ones_sb[:, :], ex[:, f0:w],
                        start=(jt == 0), stop=(jt == njt - 1))
                    nc.tensor.matmul(
                        mx[:, f0:],
                        v_sb[:, b * (L // 128) + jt,
                             h * 128:(h + 1) * 128],
                        ex[:, f0:w],
                        start=(jt == 0), stop=(jt == njt - 1))

                for h in range(HPC):
                    mx = psum.tile([128, w], F32, tag="bank",
                                   name="mx", padded_shape=[128, IC2])
                    rs = psum.tile([128, w], F32, tag="bank",
                                   name="rs", padded_shape=[128, IC2])
                    # software pipeline: colsum+AV of j-tile N are
                    # emitted behind the score of j-tile N+1, so the PE
                    # never waits on a just-started exp/mask producer
                    prev = None
                    for jt in range(njt):
                        state["counter"] += 1
                        f0 = max(0, 128 * jt - i0)
                        wl = w - f0
                        sc = psum.tile([128, w], F32, tag="bank",
                                       name="sc", padded_shape=[128, IC2])
                        nc.tensor.matmul(
                            sc[:, f0:],
                            kt_sb[:, h, b * L + jt * 128:
                                  b * L + (jt + 1) * 128],
                            qt_sb[:, h, gio + f0:gio + w],
                            start=True, stop=True)
                        ex = exp_pool.tile([128, IC2], BF16, tag="ex")
                        nc.scalar.activation(
                            ex[:, f0:w], sc[:, f0:],
                            mybir.ActivationFunctionType.Exp,
                            scale=SCALE)
                        if 128 * (jt + 1) > i0:
                            # zero where j > i
                            nc.gpsimd.affine_select(
                                ex[:, f0:w], ex[:, f0:w],
                                pattern=[[1, wl]],
                                compare_op=mybir.AluOpType.is_ge,
                                fill=0.0,
                                base=i0 + f0 - 128 * jt,
                                channel_multiplier=-1)
                        # deferred work lands between a score and the
                        # previous j-tile's colsum/AV as extra PE fill
                        drain(state["counter"])
                        if prev is not None:
                            emit_csav(h, *prev)
                        prev = (jt, f0, ex)
                        yield
                    emit_csav(h, *prev)
                    push(state["counter"] + 1, make_norm(rs, mx, mst, h, w))
                if inline_norm:
                    # free the mx/rs PSUM banks now — this chunk's msts
                    # are consumed much later (fillers of the next chunk)
                    drain(state["counter"] + 1)
                state["pending"] = (mst, obuf, gio, w)

            # ---------------- phase 1: q/k/v projections ----------------
            # the last four phase-1 chunks each pump one attention chunk
            # piecewise between their matmul groups (inputs for each are
            # complete chunks earlier): all exp/mask/norm latency hides
            # under the projection stream and exposed phase 2 shrinks
            PUMP = {5: (0, 0, 512), 6: (0, 512, 512), 7: (0, 1024, 512),
                    8: (1, 0, 256)}
            for ic, (c0, cw) in enumerate(P1):
                if ic == 3:
                    # wo is first needed at phase 2 (~190us in); issuing
                    # it here (behind chunk 4's x) keeps it out of the
                    # startup queue where it delayed x chunk 2 by ~3us
                    nc.sync.dma_start(out=wo_sb[:, 0, :], in_=wo[0])
                    nc.scalar.dma_start(out=wo_sb[:, 1, :], in_=wo[1])
                gen = None
                gsteps = 0
                if ic in PUMP:
                    pb, pi0, pw_ = PUMP[ic]
                    gen = process_chunk(pb, pi0, pw_, inline_norm=True)
                    gsteps = HPC * ((pi0 + pw_) // 128)
                ngroups = 2 * HPC + cw // 128
                pst = {"g": 0, "done": 0, "gen": gen}

                def after_group(pst=pst, gsteps=gsteps, ngroups=ngroups):
                    pst["g"] += 1
                    if pst["gen"] is None:
                        return
                    want = (pst["g"] * gsteps + ngroups - 1) // ngroups
                    while pst["done"] < want:
                        try:
                            next(pst["gen"])
                            pst["done"] += 1
                        except StopIteration:
                            pst["gen"] = None
                            break

                xch = xchs.pop(ic)
                # q^T and k^T: (o_local x i), stationary = W^T h-tiles
                ncopy = 0
                for wsb, dest in ((wq_sb, qt_sb), (wk_sb, kt_sb)):
                    for ot in range(HPC):
                        ps = psum.tile([128, cw], F32, tag="bank",
                                       name="ps",
                                       padded_shape=[128, IC1])
                        for ht in range(HT):
                            nc.tensor.matmul(
                                ps[:, :],
                                wsb[:, ht, ot * 128:(ot + 1) * 128],
                                (xch[:, :, ht, :] if cw == 512
                                 else xch[:, 0, ht, :cw]),
                                start=(ht == 0), stop=(ht == HT - 1))
                        if ncopy % 2 == 0:
                            nc.scalar.copy(
                                dest[:, ot, c0:c0 + cw], ps[:, :])
                        else:
                            nc.vector.tensor_copy(
                                dest[:, ot, c0:c0 + cw], ps[:, :])
                        ncopy += 1
                        after_group()
                        if ncopy == 1 and ic + 1 < len(P1) and (
                                ic + 1) not in xchs:
                            # prefetch emitted mid-chunk: the scheduler
                            # batches semaphore waits, so a top-of-chunk
                            # prefetch makes this chunk's first groups
                            # gate on the NEXT chunk's x delivery
                            n0, nw = P1[ic + 1]
                            xchs[ic + 1] = xpool.tile(
                                [128, 2, HT, 256], BF16,
                                tag="xch", name="xch")
                            dma_x(xchs[ic + 1], n0, nw)
                # v in natural (j x o) layout, stationary = x^T tiles
                for it in range(cw // 128):
                    ps = psum.tile([128, OPC], F32, tag="bank",
                                   name="ps", padded_shape=[128, IC1])
                    for ht in range(HT):
                        nc.tensor.matmul(
                            ps[:, :],
                            xch[:, it // 2, ht,
                                (it % 2) * 128:(it % 2 + 1) * 128],
                            wv_sb[:, ht, :],
                            start=(ht == 0), stop=(ht == HT - 1))
                    if it % 2 == 0:
                        nc.scalar.copy(
                            v_sb[:, c0 // 128 + it, :], ps[:, :])
                    else:
                        nc.vector.tensor_copy(
                            v_sb[:, c0 // 128 + it, :], ps[:, :])
                    after_group()
                if pst["gen"] is not None:
                    for _ in pst["gen"]:
                        pass

            # ---------- phase 2: attention + pipelined o-projection ----------
            # trailing (b1, 256:512) runs as two 128-wide chunks: the
            # first's o-proj drains inside the second, halving the
            # exposed final o-projection tail
            P2 = [(0, 1536, 512), (1, 512, 512), (1, 1024, 512),
                  (1, 1536, 512), (1, 256, 128), (1, 384, 128)]
            for ci, (b, i0, w) in enumerate(P2):
                for _ in process_chunk(b, i0, w, late=(ci == len(P2) - 1)):
                    pass
            # flush: remaining normalize stages, then the last chunk's
            # o-projection (copies rotated, DMAs per-2-ot, both queues)
            drain(1 << 30)
            pmst, pobuf, pgio, pw = state["pending"]
            for ot in range(H // 128):
                emit_oproj_ot(pmst, pobuf, pw, ot, tail=True)
                if ot % 2 == 1:
                    # per-2-ot DMAs alternating queues: the last exposed
                    # transfer is 128KB, not 512KB
                    g2 = ot // 2
                    iC, f0 = pgio // 512, pgio % 512
                    th = (g2 % 2) * 2
                    (nc.sync if g2 % 2 == 0 else nc.scalar).dma_start(
                        out=out[iC, g2 // 2, :, th:th + 2,
                                f0:f0 + pw],
                        in_=pobuf[:, g2 * 2:(g2 + 1) * 2, :pw])
    nc.finalize()
    return nc


_NC_CACHE = None


def _get_nc():
    global _NC_CACHE
    if _NC_CACHE is None:
        _NC_CACHE = _build()
    return _NC_CACHE


def _install_hook_shim():
    """Make antenv.axon_hooks importable (absent on this image) so
    run_bass_kernel_spmd's trace path degrades gracefully."""
    import types
    import antenv
    if "antenv.axon_hooks" not in sys.modules:
        shim = types.ModuleType("antenv.axon_hooks")

        def set_axon_ntff_profile_hook(h):
            shim._the_hook = h

        def get_axon_ntff_profile_hook():
            return getattr(shim, "_the_hook", None)

        shim.set_axon_ntff_profile_hook = set_axon_ntff_profile_hook
        shim.get_axon_ntff_profile_hook = get_axon_ntff_profile_hook
        sys.modules["antenv.axon_hooks"] = shim
        antenv.axon_hooks = shim


def _enable_profiling():
    """Wire the axon NTFF profile hook for neuron-profile timing."""
    _install_hook_shim()
    from trn_agent_boot.trn_boot import _ntff_profile_via_ctypes
    hook = _ntff_profile_via_ctypes("/opt/axon/libaxon_pjrt.so")
    sys.modules["antenv.axon_hooks"].set_axon_ntff_profile_hook(hook)
    import concourse.bass_utils as bu
    bu.upload_artifacts = lambda tmpdir: "local://" + tmpdir


def _to_bf16(a: np.ndarray):
    import ml_dtypes
    return np.ascontiguousarray(a.astype(ml_dtypes.bfloat16))


def kernel(x, padding_mask, Wq, Wk, Wv, Wo):
    global LAST_EXEC_NS
    x = np.asarray(x, dtype=np.float32)
    Wq = np.asarray(Wq, dtype=np.float32)
    Wk = np.asarray(Wk, dtype=np.float32)
    Wv = np.asarray(Wv, dtype=np.float32)
    Wo = np.asarray(Wo, dtype=np.float32)

    # packed DRAM layouts (see _build): every DMA moves 4KB contiguous
    # per-partition runs so descriptor generation never paces the kernel
    xt2d = x.reshape(BL, H).T                    # (H, BL)
    xt_p = _to_bf16(
        xt2d.reshape(2, 8, 128, 16, 256).transpose(3, 0, 2, 1, 4))
    wqt = Wq.T                                   # (H, H): [h, o]
    wkt = Wk.T
    wvt = Wv.T
    wot = Wo.T                                   # (H, H): [h_in, o]

    def pack_w(w2d):                             # (H, OPC) -> (2,128,8,256)
        return _to_bf16(
            np.ascontiguousarray(w2d).reshape(2, 8, 128, OPC)
            .transpose(0, 2, 1, 3))

    in_maps = []
    for c in range(N_CORES):
        sl = slice(c * OPC, (c + 1) * OPC)
        in_maps.append({
            "xt": xt_p,
            "wq": pack_w(wqt[:, sl]),
            "wk": pack_w(wkt[:, sl]),
            "wv": pack_w(wvt[:, sl]),
            "wo": _to_bf16(
                np.ascontiguousarray(wot[sl, :]).reshape(2, 128, H)),
        })

    profile = os.environ.get("KERNEL_PROFILE", "0") == "1"
    try:
        if profile:
            _enable_profiling()
        else:
            _install_hook_shim()
    except Exception:
        profile = False

    nc = _get_nc()
    res = run_bass_kernel_spmd(nc, in_maps, core_ids=list(range(N_CORES)),
                               trace=profile)
    LAST_EXEC_NS = res.exec_time_ns

    total = np.zeros((BL // 512, 4, 128, 4, 512), dtype=np.float32)
    for c in range(N_CORES):
        total += np.asarray(res.results[c]["out"], dtype=np.float32)
    # unpack [iC, g, p, t, f] -> out2d[g*512 + t*128 + p, iC*512 + f]
    full = total.transpose(1, 3, 2, 0, 4).reshape(H, BL)
    return np.ascontiguousarray(full.T).astype(np.float32).reshape(B, L, H)

